# revision 17
# baseline (speedup 1.0000x reference)
"""MoE grouped-GEMM expert MLP for Trainium2, expert-parallel over 8 NeuronCores.

Problem: x:(B=2, E=8, N=2048, D=1024), per-expert 2-layer GELU MLP with
w1:(E, D, F=4096), w2:(E, F, D).  Reference computes
  xe = x.reshape(E, B*N, D)          # pure buffer reinterpretation
  h  = gelu_tanh(xe @ w1 + b1)
  out= h @ w2 + b2                   # reshaped back to (B, E, N, D)

Sharding: expert parallelism — core e runs expert e on its contiguous
token block xe[e] (4096 tokens).  No collectives needed.

Per-core kernel layout trick: keep the hidden activations transposed
("hT" = [f, tok]) so both weight matrices are consumed in their NATIVE
layouts and the output comes out in natural [tok, d] layout:
  GEMM1: hT[f,tok]  = (w1[d,f] as lhsT).T @ xT[d,tok]
  GEMM2: out[tok,d] = (hT[f,tok] slice as lhsT).T @ w2[f,d]
Only x needs a transpose: chunk 0 via PE-mode fp32 transpose (PE is
otherwise idle while w1 streams in), chunks 1-7 via fp32->bf16 cast-DMA
to a DRAM scratch + hardware XBAR DMA-transpose (2-byte dtype).

Compute dtype bf16 (fp32 PSUM accumulation), gelu on ScalarE matching
jax.nn.gelu(approximate=True): end-to-end rel-err ~3.4e-3.
Measured ~0.98-1.01 ms/core on HW vs ~0.87 ms bf16 PE roofline.
"""

import numpy as np

import concourse.bacc as bacc
import concourse.mybir as mybir
import concourse.tile as tile
from concourse.bass_utils import run_bass_kernel_spmd
from concourse.masks import make_identity

E, B, N, D, F = 8, 2, 2048, 1024, 4096
TOK = B * N            # tokens per expert / per core
TC = 512               # token chunk processed per pipeline stage
NCHUNK = TOK // TC     # 8
P = 128
DO = D // P            # 8  d-tiles (GEMM1 contraction)
FO = F // P            # 32 f-tiles (GEMM2 contraction)
FG = 8                 # weight f-groups of 512 (4 f-tiles each)

F32 = mybir.dt.float32
BF16 = mybir.dt.bfloat16
GELU = mybir.ActivationFunctionType.Gelu_apprx_tanh


def _build_kernel(tc_ctx, nc, x, w1, b1, w2, b2, out):
    with (
        tc_ctx.tile_pool(name="wpool", bufs=1) as wp,
        tc_ctx.tile_pool(name="xpool", bufs=2) as xp,
        tc_ctx.tile_pool(name="hpool", bufs=1) as hp,
        tc_ctx.tile_pool(name="opool", bufs=4) as op,
        tc_ctx.tile_pool(name="cpool", bufs=1) as cp,
        tc_ctx.tile_pool(name="xf32pool", bufs=1) as xfp,
        tc_ctx.tile_pool(name="dram", bufs=1, space="DRAM") as dp,
        tc_ctx.tile_pool(name="ps1", bufs=4, space="PSUM") as ps1,
        tc_ctx.tile_pool(name="ps2", bufs=3, space="PSUM") as ps2,
        tc_ctx.tile_pool(name="psT", bufs=1, space="PSUM") as psT,
    ):
        # identity for PE-mode fp32 transpose (used for chunks 0-1)
        ident = cp.tile([P, P], F32, tag="ident")
        make_identity(nc, ident)
        # ---- load ordering matters: the gpsimd SWDGE queues drain in issue
        # order, so emit DMAs in the order compute needs them:
        #   x(chunk0) -> w1 -> x(chunk1) -> w2 -> x(chunk2..) ----

        # x cast fp32->bf16 into DRAM scratch, two tiles per chunk (halves of
        # the d axis) — few SWDGE DMAs (issue cost ~0.8us each) but fine
        # enough deps that transposes start before the whole chunk is cast.
        xb = [[None, None] for _ in range(NCHUNK)]

        def emit_x_cast(c):
            for half in range(2):
                t = dp.tile([TC, D // 2], BF16, tag=f"xb{c}_{half}")
                nc.gpsimd.dma_start(
                    t, x[c * TC:(c + 1) * TC, half * (D // 2):(half + 1) * (D // 2)]
                )
                xb[c][half] = t

        # w1 tile (ki, do, fj) = w1[do*128+ki, fg*512+fj] : lhsT for GEMM1
        w1r = w1.rearrange("(do ki) f -> ki do f", ki=P)
        # w2 tile (ki, m, dj) = w2[fg*512 + m*128 + ki, dj] : rhs for GEMM2
        w2r = w2.rearrange("(fg m ki) d -> ki fg m d", ki=P, m=4)
        w1g = [
            wp.tile([P, DO, 512], BF16, tag=f"w1g{fg}", name=f"w1g{fg}")
            for fg in range(FG)
        ]
        w2g = [
            wp.tile([P, 4, D], BF16, tag=f"w2g{fg}", name=f"w2g{fg}")
            for fg in range(FG)
        ]

        def emit_w1(fgs):
            for fg in fgs:
                nc.gpsimd.dma_start(w1g[fg], w1r[:, :, fg * 512:(fg + 1) * 512])

        def emit_w2(fgs):
            for fg in fgs:
                nc.gpsimd.dma_start(w2g[fg], w2r[:, fg])

        # SWDGE queue order ~= consumption order: GEMM1-c0 eats w1 groups
        # from ~20us, GEMM2-c0 eats w2 groups from ~75us, chunk c's x is
        # needed at ~(110*c)us.  Chunk 0's x bypasses SWDGE entirely (fp32
        # HWDGE load + PE transpose below), so SWDGE starts on w1 at t~10us.
        emit_w1(range(8))
        emit_x_cast(1)
        emit_w2(range(8))
        for c in range(2, NCHUNK):
            emit_x_cast(c)

        # chunk 0 x: fp32 load on the (otherwise idle) sync HWDGE queue
        xf32 = xfp.tile([P, 4, D], F32, tag="xf32")
        nc.sync.dma_start(xf32, x[0:TC, :].rearrange("(tm p) d -> p tm d", p=P))

        # ---- biases ----
        # b1 on partitions (f-inner), one column per f-tile -> activation bias
        b1sb = cp.tile([P, FO], F32, tag="b1")
        nc.sync.dma_start(b1sb, b1.rearrange("(fo fi) -> fi fo", fi=P))
        # b2 replicated across all 128 partitions (free dim = d)
        b2sb = cp.tile([P, D], F32, tag="b2")
        nc.sync.dma_start(b2sb[0:1, :], b2[None, :])
        k = 1
        while k < P:
            nc.sync.dma_start(b2sb[k:2 * k, :], b2sb[0:k, :])
            k *= 2

        # ---- main pipeline over token chunks ----
        for c in range(NCHUNK):
            xT = xp.tile([P, DO, TC], BF16, tag="xT")
            if c == 0:
                # PE-mode fp32 transpose straight from the HWDGE fp32 load:
                # keeps chunk 0 off the SWDGE queues so w1 streams from t=0,
                # and uses the PE while it would otherwise wait for w1.
                for tm in range(TC // P):
                    for do in range(DO):
                        pt = psT.tile([P, P], F32, tag="psT")
                        nc.tensor.transpose(
                            pt, xf32[:, tm, do * P:(do + 1) * P], ident
                        )
                        nc.vector.tensor_copy(
                            xT[:, do, tm * P:(tm + 1) * P], pt
                        )
            else:
                # xT[d-part, do, tok] via XBAR DMA transpose
                for do in range(DO):
                    src = xb[c][do // 4]
                    nc.sync.dma_start_transpose(
                        xT[:, do, :], src[:, (do % 4) * P:(do % 4 + 1) * P]
                    )

            # GEMM1 + bias + gelu -> hT[f-part, fo, tok] (bf16)
            hT = hp.tile([P, FO, TC], BF16, tag="hT")
            for fo in range(FO):
                ps = ps1.tile([P, TC], F32, tag="ps1")
                w1t = w1g[fo // 4]
                fi = (fo % 4) * P
                for do in range(DO):
                    nc.tensor.matmul(
                        ps,
                        w1t[:, do, fi:fi + P],
                        xT[:, do, :],
                        start=(do == 0),
                        stop=(do == DO - 1),
                    )
                nc.scalar.activation(
                    hT[:, fo, :], ps, GELU, bias=b1sb[:, fo:fo + 1]
                )

            # GEMM2 + bias -> out[tok, d] natural layout
            for tt in range(TC // P):
                for dh in range(2):
                    ps2t = ps2.tile([P, 512], F32, tag="ps2")
                    for fo in range(FO):
                        nc.tensor.matmul(
                            ps2t,
                            hT[:, fo, tt * P:(tt + 1) * P],
                            w2g[fo // 4][:, fo % 4, dh * 512:(dh + 1) * 512],
                            start=(fo == 0),
                            stop=(fo == FO - 1),
                        )
                    osb = op.tile([P, 512], F32, tag="osb")
                    nc.vector.tensor_tensor(
                        osb, ps2t, b2sb[:, dh * 512:(dh + 1) * 512],
                        mybir.AluOpType.add,
                    )
                    row0 = c * TC + tt * P
                    nc.sync.dma_start(
                        out[row0:row0 + P, dh * 512:(dh + 1) * 512], osb
                    )


_NC_CACHE = None


def _get_nc():
    global _NC_CACHE
    if _NC_CACHE is None:
        nc = bacc.Bacc(
            "TRN2", target_bir_lowering=False, num_devices=E, num_swdge_queues=4
        )
        x = nc.dram_tensor("x", [TOK, D], F32, kind="ExternalInput").ap()
        w1 = nc.dram_tensor("w1", [D, F], F32, kind="ExternalInput").ap()
        b1 = nc.dram_tensor("b1", [F], F32, kind="ExternalInput").ap()
        w2 = nc.dram_tensor("w2", [F, D], F32, kind="ExternalInput").ap()
        b2 = nc.dram_tensor("b2", [D], F32, kind="ExternalInput").ap()
        out = nc.dram_tensor("out", [TOK, D], F32, kind="ExternalOutput").ap()
        with tile.TileContext(nc) as tctx:
            _build_kernel(tctx, nc, x, w1, b1, w2, b2, out)
        nc.compile()
        _NC_CACHE = nc
    return _NC_CACHE


def kernel(run_opts=None, **inputs):
    x = np.ascontiguousarray(inputs["x"], dtype=np.float32)
    w1 = np.ascontiguousarray(inputs["w1"], dtype=np.float32)
    b1 = np.ascontiguousarray(inputs["b1"], dtype=np.float32)
    w2 = np.ascontiguousarray(inputs["w2"], dtype=np.float32)
    b2 = np.ascontiguousarray(inputs["b2"], dtype=np.float32)

    # x.view(E, B, N, D) in the reference is a pure reshape: expert e owns the
    # contiguous token block e of the flattened (E*B*N, D) buffer.
    xf = x.reshape(E, TOK, D)
    in_maps = [
        {"x": xf[e], "w1": w1[e], "b1": b1[e], "w2": w2[e], "b2": b2[e]}
        for e in range(E)
    ]
    nc = _get_nc()
    res = run_bass_kernel_spmd(
        nc, in_maps, core_ids=list(range(E)), **(run_opts or {})
    )
    outs = np.stack([res.results[e]["out"] for e in range(E)])  # (E, TOK, D)
    if run_opts:
        kernel.last_results = res
    # outputs.view(B, E, N, D) in the reference: reinterpret (E, B*N, D) buffer
    return outs.reshape(B, E, N, D)
